# revision 50
# baseline (speedup 1.0000x reference)
"""LoRA self-attention TRN2 kernel (8 NeuronCores, SPMD) — v5.

Sharding: core c = (b, hp) with b = c // 4 (batch), hp = c % 4 (head group of
4 heads = 256 channels). Each core computes q/k/v projections (+LoRA) for its
256 output channels from the full x[b], runs attention for its 4 heads, and a
partial output projection over its 256 context channels. Host sums the 4
partials per batch element and adds bo.

Numerics: q/k projections and the [k,q]-oriented QK^T scores use bf16 hi/lo
splits (s = kh·qh + kl·qh + kh·ql, fp32-grade); the softmax shift m-hat comes
from a single-bf16 [q,k] score pass (error ≪ the exp-safety slack; the shift
cancels exactly in softmax). P·V and the output projection run in bf16.

v5 scheduling (the PE queue is strictly in-order, so long PE->DVE ping-pong
chains must be interleaved at fine grain with independent matmuls):
  - m-hat matmuls+reduces are emitted one per score-tile iteration (and
    threaded through the k/v projection loops for head 0), so the PE never
    idles behind a DVE reduce_max and the HAM clock gate stays warm.
  - x is DMA'd in ns-major 512-column slices, after the q-projection weights,
    so the first projection group starts ~8us in instead of ~27us.
  - v is computed directly in [T, O] orientation (no PE transposes).
  - the output projection + its DMA are interleaved into head 3's score loop
    (reusing the idle m-hat PSUM banks), removing the serial tail.
  - PV accumulators are evacuated to SBUF immediately so the next PV group
    never stalls behind the softmax-normalize chain.
  - ones-column on V makes PV row 64 the softmax normalizer Z (no reduce);
    m-hat lands in qla row 64 via a DRAM transpose bounce so the K=65 score
    matmul subtracts it inside PSUM for free.
  - when every LoRA B factor is zero (standard LoRA init), a specialized
    no-LoRA program is compiled and used; the general path handles B != 0.
"""
import sys

sys.path.insert(0, "/opt/trn_rl_repo")

from contextlib import ExitStack

import numpy as np
import ml_dtypes

import concourse.bass as bass
import concourse.tile as tile
from concourse import bacc, mybir
from concourse.bass import ts
from concourse.bass_utils import run_bass_kernel_spmd

F32 = mybir.dt.float32
BF16 = mybir.dt.bfloat16
bf16 = ml_dtypes.bfloat16
AX = mybir.AxisListType
Exp = mybir.ActivationFunctionType.Exp

T = 2048          # sequence length
E = 1024          # embed
OL = 256          # local output channels (4 heads)
D = 64            # head dim
NH = 4            # local heads
R = 8             # lora rank
CI = 8            # contraction chunks of 128 over E
NS = 4            # 512-wide slices over T
TC = 16           # 128-wide tiles over T
VW = 65           # v-aug width per head (64 + ones column)

_CACHE = {}


def _build(lora=True):
    key = ("nc", lora)
    if key in _CACHE:
        return _CACHE[key]

    nc = bacc.Bacc("TRN2", target_bir_lowering=False, debug=False)

    # ---- DRAM I/O ----
    xth_d = nc.dram_tensor("xth", [E, T], BF16, kind="ExternalInput")
    xtl_d = nc.dram_tensor("xtl", [E, T], BF16, kind="ExternalInput")
    w_d = {}
    for p in "qkv":
        for s in "hl":
            if p == "v" and s == "l":
                continue
            w_d[p + s] = nc.dram_tensor(f"w{p}{s}", [E, OL], BF16, kind="ExternalInput")
    woT_d = nc.dram_tensor("woT", [OL, E], BF16, kind="ExternalInput")
    if lora:
        ah_d = nc.dram_tensor("ah", [E, 3 * R], BF16, kind="ExternalInput")
        b_d = {p: nc.dram_tensor(f"b{p}", [R, OL], BF16, kind="ExternalInput")
               for p in "qkv"}
    ident_d = nc.dram_tensor("ident", [128, 128], BF16, kind="ExternalInput")
    outp_d = nc.dram_tensor("outp", [T, E], BF16, kind="ExternalOutput")

    with tile.TileContext(nc) as tc, ExitStack() as ctx:
        # ---------------- persistent tiles ----------------
        # Per-head score operand layouts:
        #   khl[h] [128,T]: rows 0:64 = kT_hi(h), rows 64:128 = kT_lo(h)
        #   kha[h] [65,T]:  rows 0:64 = kT_hi(h), row 64 = ones
        #   qhh[h] [128,T]: rows 0:64 = qT_hi(h), rows 64:128 = qT_hi(h) (dup)
        #   qla[h] [65,T]:  rows 0:64 = qT_lo(h), row 64 = -m-hat
        pers = ctx.enter_context(tc.tile_pool(name="pers", bufs=1))
        khl = [pers.tile([128, T], BF16, name=f"khl{h}") for h in range(NH)]
        kha = [pers.tile([65, T], BF16, name=f"kha{h}") for h in range(NH)]
        # kh duplicated onto partitions 64:128 (no-lora only): lets m-hat
        # matmuls run as row-tiled PAIRS — two K=64 matmuls in PE row groups
        # (0,0)/(64,0) execute concurrently, halving the m-hat pass's time
        khh = (None if lora else
               [pers.tile([128, T], BF16, name=f"khh{h}") for h in range(NH)])
        qhh = [pers.tile([128, T], BF16, name=f"qhh{h}") for h in range(NH)]
        qla = [pers.tile([65, T], BF16, name=f"qla{h}") for h in range(NH)]
        v16 = [pers.tile([128, NH * VW], BF16, name=f"v16_{i}") for i in range(TC)]
        ident = pers.tile([128, 128], BF16, name="ident")
        ctxT_t = [pers.tile([128, T], BF16, name=f"ctxT{c}") for c in range(2)]

        # ---------------- attention-lifetime pools ----------------
        # (ptp/ost_p and the PSUM pools enter after phase 1 so their space
        # reuses the x tiles' / projection pools')
        att = ctx.enter_context(tc.tile_pool(name="att", bufs=2))
        drp = ctx.enter_context(tc.tile_pool(name="drp", bufs=2, space="DRAM"))

        woT_t = [pers.tile([128, E], BF16, name=f"woT{cc}") for cc in range(2)]

        # ---------------- m-hat machinery (emitted interleaved) ----------
        # mh_step(h, i, pool), i in 0..63: one single-bf16 [q,k] score matmul
        # (qt = i//4 stationary, k-slice i%4 moving) + row-max reduce.
        # mh_finish(h, pool): merge quarter maxes, negate, PE-transpose, DRAM
        # bounce into qla[h] row 64.
        rm4 = {}

        def mh_step(h, i, pool):
            qt, quarter = i // 4, i % 4
            if quarter == 0 and qt == 0:
                rm4[h] = [att.tile([128, 16], F32, tag=f"rm4{q}", name=f"rm4_{h}{q}")
                          for q in range(4)]
            if lora or i % 2 == 0:
                ms = pool.tile([128, 512], F32, tag="ms", name="ms")
                nc.tensor.matmul(ms, qhh[h][0:64, ts(qt, 128)],
                                 khl[h][0:64, ts(quarter, 512)],
                                 start=True, stop=True)
            else:
                # odd steps ride PE row group (64,0) concurrently with the
                # preceding even step's matmul in rows 0:63
                ms = pool.tile([128, 512], F32, tag="ms", name="ms")
                nc.tensor.matmul(ms, qhh[h][64:128, ts(qt, 128)],
                                 khh[h][64:128, ts(quarter, 512)],
                                 start=True, stop=True)
            nc.vector.reduce_max(out=rm4[h][quarter][:, qt:qt + 1], in_=ms, axis=AX.X)

        rm16s_t = {}

        def mh_merge(h):
            # DVE-only: merge quarter maxes and negate
            r = rm4[h]
            ra = att.tile([128, 16], F32, name="ra")
            rb = att.tile([128, 16], F32, name="rb")
            nc.vector.tensor_max(ra, r[0], r[1])
            nc.vector.tensor_max(rb, r[2], r[3])
            rm16 = att.tile([128, 16], F32, name="rm16")
            nc.vector.tensor_max(rm16, ra, rb)
            rm16s = att.tile([128, 16], BF16, tag="rm16s", name="rm16s")
            nc.vector.tensor_scalar_mul(rm16s, rm16, -1.0)
            rm16s_t[h] = rm16s

        def mh_bounce(h, pool):
            # transpose on PE, then a burst-contiguous DRAM bounce:
            # qla[h][64, qt*128+q] = rm16s[q, qt]  (emitted well after
            # mh_merge so the PE transpose never waits on the DVE chain)
            mtr = pool.tile([16, 128], BF16, tag="ms", name="mtr")
            nc.tensor.transpose(mtr, rm16s_t[h], ident)
            rmT = att.tile([16, 128], BF16, name="rmT")
            nc.vector.tensor_copy(rmT, mtr)
            dr = drp.tile([16, 128], BF16, name="mh_dr")
            nc.sync.dma_start(out=dr, in_=rmT)
            src = bass.AP(tensor=dr.tensor, offset=dr.offset, ap=[[1, 16 * 128]])
            nc.sync.dma_start(out=qla[h][64:65, :], in_=src)

        def mh_finish(h, pool):
            mh_merge(h)
            mh_bounce(h, pool)

        # ---------------- phase 1: projections ----------------
        with ExitStack() as ph1:
            ld = ph1.enter_context(tc.tile_pool(name="ld", bufs=1))
            wpool = ph1.enter_context(
                tc.tile_pool(name="wpool", bufs=1 if lora else 2))
            pps = ph1.enter_context(
                tc.tile_pool(name="pps", bufs=1 if lora else 2, space="PSUM"))
            msp0 = ph1.enter_context(tc.tile_pool(name="msp0", bufs=4, space="PSUM"))
            if lora:
                upsp = ph1.enter_context(
                    tc.tile_pool(name="upsp", bufs=1, space="PSUM"))

            for h in range(NH):
                nc.vector.memset(kha[h][64:65, :], 1.0)
            for tci in range(TC):
                nc.vector.memset(v16[tci], 1.0)



            # one consolidated DMA per (weight, half): [128, ci, OL] pattern
            wt = {}

            # hi-half DMAs go on the Sync queue, lo-half on GpSimd's, so the
            # two serial DMA chains run concurrently and the first projection
            # group (wh·xh) is gated only by the short Sync chain
            def load_w_half(p, s):
                wa = wpool.tile([128, CI, OL], BF16, tag=f"w{s}", name=f"w{p}{s}")
                src = bass.AP(tensor=w_d[p + s], offset=0,
                              ap=[[OL, 128], [128 * OL, CI], [1, OL]])
                nc.sync.dma_start(out=wa, in_=src)
                return wa

            def load_w(p):
                wt[p] = [load_w_half(p, "h"),
                         None if p == "v" else load_w_half(p, "l")]

            # x arrives in ns-major 512-column slices (one DMA per slice
            # covering all 16 row-chunks). DMAs are serial on the Sync queue,
            # so issue exactly in first-use order: wq-hi, x-hi ns0 (the first
            # projection group's 8 wh·xh matmuls), then wq-lo, x-lo ns0, ...
            xth_a = ld.tile([128, CI, T], BF16, name="xth_a")
            xtl_a = ld.tile([128, CI, T], BF16, name="xtl_a")

            def load_x(dst, dram, ns, half=None):
                ci0, cin = (0, CI) if half is None else (half * CI // 2, CI // 2)
                src = bass.AP(tensor=dram, offset=ci0 * 128 * T + ns * 512,
                              ap=[[T, 128], [128 * T, cin], [1, 512]])
                nc.sync.dma_start(out=dst[:, ci0:ci0 + cin, ts(ns, 512)], in_=src)

            # DMA issue in exact first-use order (serial Sync queue): wq-hi,
            # x-hi ns0 (halved so the first 4 matmuls start sooner), x-lo ns0,
            # then wq-lo (its wl·xh consumers come after the xl ones)
            wqh = load_w_half("q", "h")
            load_x(xth_a, xth_d, 0, half=0)
            load_x(xth_a, xth_d, 0, half=1)
            load_x(xtl_a, xtl_d, 0, half=0)
            load_x(xtl_a, xtl_d, 0, half=1)
            wql = load_w_half("q", "l")
            wt["q"] = [wqh, wql]
            for ns in range(1, NS):
                load_x(xth_a, xth_d, ns)
                load_x(xtl_a, xtl_d, ns)
            xth_t = [xth_a[:, ci, :] for ci in range(CI)]
            xtl_t = [xtl_a[:, ci, :] for ci in range(CI)]

            u_bf = {}
            b_t = {}
            if lora:
                # single-bf16 LoRA: A/B factors are tiny additive corrections,
                # so one bf16 pass is well within the error budget
                ah_t = []
                for ci in range(CI):
                    t_ = ld.tile([128, 3 * R], BF16, name=f"ah{ci}")
                    nc.sync.dma_start(out=t_, in_=ah_d[ts(ci, 128), :])
                    ah_t.append(t_)
                for key2, d in b_d.items():
                    t_ = ld.tile([R, OL], BF16, name=f"b{key2}")
                    nc.sync.dma_start(out=t_, in_=d[:, :])
                    b_t[key2] = t_

                # u_all = x @ A_all, shared M=24 pass
                uf = ld.tile([3 * R, T], F32, name="uf")
                for ns in range(NS):
                    sl = ts(ns, 512)
                    ups = upsp.tile([3 * R, 512], F32, tag="ups", name="ups")
                    n_mm = 2 * CI
                    i = 0
                    for ci in range(CI):
                        for x_t in (xth_t[ci], xtl_t[ci]):
                            nc.tensor.matmul(ups, ah_t[ci], x_t[:, sl],
                                             start=(i == 0), stop=(i == n_mm - 1))
                            i += 1
                    nc.any.tensor_copy(uf[:, sl], ups)
                for pi, p in enumerate("qkv"):
                    upf = ld.tile([R, T], F32, tag="upf", name=f"u{p}f")
                    nc.sync.dma_start(out=upf, in_=uf[pi * R:(pi + 1) * R, :])
                    uh = ld.tile([R, T], BF16, name=f"u{p}h")
                    nc.vector.tensor_copy(uh, upf)
                    u_bf[p] = uh

            # --- q/k projections, transposed layout [OL, T] ---
            # MM sequence is kind-major (wh·xh first) so the first group can
            # start before xtl's DMA lands
            def qk_proj(p, oc, mh_per_ns=0, mh_base=0):
                wh_a, wl_a = wt[p]
                osl = slice(oc * 128, oc * 128 + 128)
                h0, h1 = 2 * oc, 2 * oc + 1
                for ns in range(NS):
                    sl = ts(ns, 512)
                    ps = pps.tile([128, 512], F32, tag="proj", name="proj")
                    seq = [(wh_a[:, ci, osl], xth_t[ci][:, sl]) for ci in range(CI)]
                    seq += [(wh_a[:, ci, osl], xtl_t[ci][:, sl]) for ci in range(CI)]
                    seq += [(wl_a[:, ci, osl], xth_t[ci][:, sl]) for ci in range(CI)]
                    if lora:
                        seq += [(b_t[p][:, osl], u_bf[p][:, sl])]
                    for i, (a, b_) in enumerate(seq):
                        nc.tensor.matmul(ps, a, b_, start=(i == 0),
                                         stop=(i == len(seq) - 1))
                    if p == "q":
                        for h, rows in ((h0, ps[0:64, :]), (h1, ps[64:128, :])):
                            nc.any.tensor_copy(qhh[h][0:64, sl], rows)
                            nc.any.tensor_copy(qhh[h][64:128, sl], rows)
                            nc.vector.tensor_sub(qla[h][0:64, sl], rows,
                                                 qhh[h][0:64, sl])
                    else:
                        for h, rows in ((h0, ps[0:64, :]), (h1, ps[64:128, :])):
                            nc.any.tensor_copy(khl[h][0:64, sl], rows)
                            nc.any.tensor_copy(kha[h][0:64, sl], rows)
                            nc.vector.tensor_sub(khl[h][64:128, sl], rows,
                                                 khl[h][0:64, sl])
                            if not lora:
                                nc.any.tensor_copy(khh[h][64:128, sl], rows)
                    for j in range(mh_per_ns):
                        mh_step(0, mh_base + ns * mh_per_ns + j, msp0)

            if lora:
                # wpool bufs=1 in the lora build: finish all readers of a
                # weight before loading the next (else buffer-rotation WAR
                # deadlocks against the in-order PE queue)
                qk_proj("q", 0)
                qk_proj("q", 1)
                load_w("k")
                qk_proj("k", 0)
                qk_proj("k", 1, mh_per_ns=8, mh_base=0)
                load_w("v")
                v_mh, v_base = 2, 32
            else:
                qk_proj("q", 0)
                load_w("k")
                qk_proj("k", 0)
                # mh(0) needs qhh[0]/khl[0] (ready after q/k oc0): spread its
                # 64 steps through q-oc1 (32), k-oc1 (24) and v tci 0..7 (8)
                # so the DVE reduce chain finishes alongside the v projection
                qk_proj("q", 1, mh_per_ns=8, mh_base=0)
                qk_proj("k", 1, mh_per_ns=6, mh_base=32)
                load_w("v")
                v_mh, v_base = 2, 56

            # ident / woT are needed late; issue after all projection weights
            # (runs in both lora and no-lora builds)
            nc.sync.dma_start(out=ident, in_=ident_d[:, :])
            for cc in range(2):
                nc.sync.dma_start(out=woT_t[cc], in_=woT_d[ts(cc, 128), :])

            # --- v directly in [T, O] orientation (no transposes) ---
            wvh = wt["v"][0]
            for tci in range(TC):
                tsl = ts(tci, 128)
                ps = pps.tile([128, OL], F32, tag="proj", name="proj")
                seq = [(xth_t[ci][:, tsl], wvh[:, ci, :]) for ci in range(CI)]
                if lora:
                    seq += [(u_bf["v"][:, tsl], b_t["v"][:, :])]
                for i, (a, b_) in enumerate(seq):
                    nc.tensor.matmul(ps, a, b_, start=(i == 0),
                                     stop=(i == len(seq) - 1))
                for h in range(NH):
                    nc.any.tensor_copy(v16[tci][:, h * VW:h * VW + 64],
                                       ps[:, h * 64:(h + 1) * 64])
                i_mh = v_base + tci * v_mh
                for j in range(v_mh):
                    if i_mh + j < 64:
                        mh_step(0, i_mh + j, msp0)
                if i_mh + v_mh >= 64 and i_mh < 64:
                    mh_finish(0, msp0)

        # ---------------- phase 3: attention ----------------
        ptp = ctx.enter_context(tc.tile_pool(name="ptp", bufs=2))
        ost_p = ctx.enter_context(tc.tile_pool(name="ost", bufs=3))
        sps = ctx.enter_context(
            tc.tile_pool(name="sps", bufs=2 if lora else 3, space="PSUM"))
        msp = ctx.enter_context(tc.tile_pool(name="msp", bufs=4, space="PSUM"))
        cps = ctx.enter_context(tc.tile_pool(name="cps", bufs=1, space="PSUM"))

        # outproj(tci): emitted interleaved into head 3's loop
        ops_state = {}

        def outproj_mm(tci, no):
            tsl = ts(tci, 128)
            op_t = msp.tile([128, 512], F32, tag="ms", name="op")
            for cc in range(2):
                nc.tensor.matmul(op_t, ctxT_t[cc][:, tsl], woT_t[cc][:, ts(no, 512)],
                                 start=(cc == 0), stop=(cc == 1))
            if no == 0:
                ops_state[tci] = ost_p.tile([128, E], BF16, tag="ost", name="ost")
            ost = ops_state[tci]
            nc.vector.tensor_copy(ost[:, ts(no, 512)], op_t)
            # per-half store so the final DMA isn't gated on both casts
            nc.sync.dma_start(out=outp_d[tsl, ts(no, 512)], in_=ost[:, ts(no, 512)])

        def outproj_steps(qb):
            # 8 paired-MM slots per score loop: 4 tci x 2 no
            return [(tci, no) for tci in range(qb * 4, qb * 4 + 4)
                    for no in range(2)]

        for h in range(NH):
            ch = h // 2
            pr = (h % 2) * 64
            mh_count = [0]
            for qb in range(NS):
                qsl = ts(qb, 512)
                # --- sT pass: K-stacked scores with fused -m-hat -> exp ---
                # one mh(h+1) step (or outproj MM for h==3) per kt so the PE
                # queue never stalls behind the DVE reduce chain
                steps = outproj_steps(qb - 1) if (h == 3 and qb > 0) else None
                pT = [ptp.tile([128, 512], BF16, tag=f"pt{i}", name=f"pt{i}")
                      for i in range(TC)]
                cxa = cps.tile([VW, 512], F32, tag="cxa", name="cxa")

                def pv_mm(kt):
                    nc.tensor.matmul(cxa, v16[kt][:, h * VW:(h + 1) * VW], pT[kt],
                                     start=(kt == 0), stop=(kt == TC - 1),
                                     skip_group_check=True)

                for kt in range(TC):
                    st = sps.tile([128, 512], F32, tag="st", name="st")
                    # kh·qh + kl·qh in one K=128 matmul (qh duplicated)
                    nc.tensor.matmul(st, khl[h][:, ts(kt, 128)], qhh[h][:, qsl],
                                     start=True, stop=False)
                    # kh·ql + ones·(-m-hat), K=65
                    nc.tensor.matmul(st, kha[h][:, ts(kt, 128)],
                                     qla[h][:, qsl], start=False, stop=True)
                    nc.scalar.activation(out=pT[kt], in_=st, func=Exp, scale=0.125)
                    # PV interleaved at lag 2 so cxa completes with the score
                    # loop and the normalize chain starts immediately
                    if kt >= 2:
                        pv_mm(kt - 2)
                    if h < NH - 1 and qb < 3:
                        # 64 mh(h+1) steps over qb0..2 so the bounce lands
                        # during qb3, before head h+1 needs it; no-lora emits
                        # row-tiled pairs back-to-back so they overlap on PE
                        target = ((qb * TC + kt + 1) * 64) // (3 * TC)
                        if not lora:
                            target &= ~1
                        while mh_count[0] < target:
                            mh_step(h + 1, mh_count[0], msp)
                            mh_count[0] += 1
                    elif steps is not None and kt % 2 == 0:
                        outproj_mm(*steps[kt // 2])
                    if h < NH - 1 and qb == 3 and kt == 8:
                        mh_bounce(h + 1, msp)
                pv_mm(TC - 2)
                pv_mm(TC - 1)
                if h == NH - 1 and qb == NS - 1:
                    # keep the PE busy through the final normalize chain
                    # (a >3.4us idle window re-throttles the HAM clock gate,
                    # halving the closing output-projection matmuls)
                    for _ in range(10):
                        ms = msp.tile([128, 512], F32, tag="ms", name="fill")
                        nc.tensor.matmul(ms, khl[0][:, 0:128], qhh[0][:, 0:512],
                                         start=True, stop=True)
                if h < NH - 1 and qb == 2:
                    mh_merge(h + 1)
                # evacuate PSUM immediately so the next PV group never waits
                cxs = att.tile([VW, 512], F32, tag="cxs", name="cxs")
                nc.vector.tensor_copy(cxs, cxa)
                # --- normalize by Z (row 64) off the critical path ---
                zrow = att.tile([1, 512], F32, name="zrow")
                nc.vector.tensor_copy(zrow, cxs[64:65, :])
                rcpz = att.tile([1, 512], F32, name="rcpz")
                nc.vector.reciprocal_approx_fast(out=rcpz, in_=zrow)
                rcp_bc = att.tile([64, 512], F32, name="rcp_bc")
                nc.gpsimd.partition_broadcast(rcp_bc, rcpz, channels=64)
                nc.vector.tensor_mul(ctxT_t[ch][pr:pr + 64, qsl], cxs[0:64, :],
                                     rcp_bc)

        # ---------------- tail: last output-projection block ----------------
        for tci, no in outproj_steps(NS - 1):
            outproj_mm(tci, no)

    nc.compile()
    _CACHE[key] = nc
    return nc


def _split(a):
    h = a.astype(bf16)
    l = (a - h.astype(np.float32)).astype(bf16)
    return h, l


def _shard(inputs, lora):
    x = np.asarray(inputs["x"], np.float32)
    Wo = np.asarray(inputs["Wo"], np.float32)
    ident = np.eye(128, dtype=np.float32).astype(bf16)
    if lora:
        A_all = np.concatenate([np.asarray(inputs["Aq"], np.float32),
                                np.asarray(inputs["Ak"], np.float32),
                                np.asarray(inputs["Av"], np.float32)], axis=1)
        ah = A_all.astype(bf16)
    in_maps = []
    for core in range(8):
        b, hp = core // 4, core % 4
        o0 = hp * OL
        xT = np.ascontiguousarray(x[b].T)
        xh, xl = _split(xT)
        m = {"xth": xh, "xtl": xl, "ident": ident}
        for p in "qkv":
            W = np.asarray(inputs["W" + p], np.float32)
            Ws = np.ascontiguousarray(W[o0:o0 + OL, :].T)
            wh, wl = _split(Ws)
            m["w%sh" % p] = wh
            if p != "v":
                m["w%sl" % p] = wl
            if lora:
                B = np.asarray(inputs["B" + p], np.float32)[:, o0:o0 + OL] * 2.0
                m["b" + p] = B.astype(bf16)
        m["woT"] = np.ascontiguousarray(Wo[:, o0:o0 + OL].T).astype(bf16)
        if lora:
            m["ah"] = ah
        in_maps.append(m)
    return in_maps


def _run(inputs, trace=False, **kw):
    lora = not all(
        np.count_nonzero(np.asarray(inputs["B" + p])) == 0 for p in "qkv")
    nc = _build(lora)
    in_maps = _shard(inputs, lora)
    res = run_bass_kernel_spmd(nc, in_maps, core_ids=list(range(8)), trace=trace, **kw)
    bo = np.asarray(inputs["bo"], np.float32)
    parts = [res.results[c]["outp"].astype(np.float64) for c in range(8)]
    out = np.stack([sum(parts[0:4]), sum(parts[4:8])]) + bo.astype(np.float64)
    return out.astype(np.float32), res


def kernel(**inputs):
    out, _ = _run(inputs)
    return out


# revision 51
# speedup vs baseline: 1.0079x; 1.0079x over previous
"""LoRA self-attention TRN2 kernel (8 NeuronCores, SPMD) — v5.

Sharding: core c = (b, hp) with b = c // 4 (batch), hp = c % 4 (head group of
4 heads = 256 channels). Each core computes q/k/v projections (+LoRA) for its
256 output channels from the full x[b], runs attention for its 4 heads, and a
partial output projection over its 256 context channels. Host sums the 4
partials per batch element and adds bo.

Numerics: q/k projections and the [k,q]-oriented QK^T scores use bf16 hi/lo
splits (s = kh·qh + kl·qh + kh·ql, fp32-grade); the softmax shift m-hat comes
from a single-bf16 [q,k] score pass (error ≪ the exp-safety slack; the shift
cancels exactly in softmax). P·V and the output projection run in bf16.

v5 scheduling (the PE queue is strictly in-order, so long PE->DVE ping-pong
chains must be interleaved at fine grain with independent matmuls):
  - m-hat matmuls+reduces are emitted one per score-tile iteration (and
    threaded through the k/v projection loops for head 0), so the PE never
    idles behind a DVE reduce_max and the HAM clock gate stays warm.
  - x is DMA'd in ns-major 512-column slices, after the q-projection weights,
    so the first projection group starts ~8us in instead of ~27us.
  - v is computed directly in [T, O] orientation (no PE transposes).
  - the output projection + its DMA are interleaved into head 3's score loop
    (reusing the idle m-hat PSUM banks), removing the serial tail.
  - PV accumulators are evacuated to SBUF immediately so the next PV group
    never stalls behind the softmax-normalize chain.
  - ones-column on V makes PV row 64 the softmax normalizer Z (no reduce);
    m-hat lands in qla row 64 via a DRAM transpose bounce so the K=65 score
    matmul subtracts it inside PSUM for free.
  - when every LoRA B factor is zero (standard LoRA init), a specialized
    no-LoRA program is compiled and used; the general path handles B != 0.
"""
import sys

sys.path.insert(0, "/opt/trn_rl_repo")

from contextlib import ExitStack

import numpy as np
import ml_dtypes

import concourse.bass as bass
import concourse.tile as tile
from concourse import bacc, mybir
from concourse.bass import ts
from concourse.bass_utils import run_bass_kernel_spmd

F32 = mybir.dt.float32
BF16 = mybir.dt.bfloat16
bf16 = ml_dtypes.bfloat16
AX = mybir.AxisListType
Exp = mybir.ActivationFunctionType.Exp

T = 2048          # sequence length
E = 1024          # embed
OL = 256          # local output channels (4 heads)
D = 64            # head dim
NH = 4            # local heads
R = 8             # lora rank
CI = 8            # contraction chunks of 128 over E
NS = 4            # 512-wide slices over T
TC = 16           # 128-wide tiles over T
VW = 65           # v-aug width per head (64 + ones column)

_CACHE = {}


def _build(lora=True):
    key = ("nc", lora)
    if key in _CACHE:
        return _CACHE[key]

    nc = bacc.Bacc("TRN2", target_bir_lowering=False, debug=False)

    # ---- DRAM I/O ----
    xth_d = nc.dram_tensor("xth", [E, T], BF16, kind="ExternalInput")
    xtl_d = nc.dram_tensor("xtl", [E, T], BF16, kind="ExternalInput")
    w_d = {}
    for p in "qkv":
        for s in "hl":
            if p == "v" and s == "l":
                continue
            w_d[p + s] = nc.dram_tensor(f"w{p}{s}", [E, OL], BF16, kind="ExternalInput")
    woT_d = nc.dram_tensor("woT", [OL, E], BF16, kind="ExternalInput")
    if lora:
        ah_d = nc.dram_tensor("ah", [E, 3 * R], BF16, kind="ExternalInput")
        b_d = {p: nc.dram_tensor(f"b{p}", [R, OL], BF16, kind="ExternalInput")
               for p in "qkv"}
    ident_d = nc.dram_tensor("ident", [128, 128], BF16, kind="ExternalInput")
    outp_d = nc.dram_tensor("outp", [T, E], BF16, kind="ExternalOutput")

    with tile.TileContext(nc) as tc, ExitStack() as ctx:
        # ---------------- persistent tiles ----------------
        # Per-head score operand layouts:
        #   khl[h] [128,T]: rows 0:64 = kT_hi(h), rows 64:128 = kT_lo(h)
        #   kha[h] [65,T]:  rows 0:64 = kT_hi(h), row 64 = ones
        #   qhh[h] [128,T]: rows 0:64 = qT_hi(h), rows 64:128 = qT_hi(h) (dup)
        #   qla[h] [65,T]:  rows 0:64 = qT_lo(h), row 64 = -m-hat
        pers = ctx.enter_context(tc.tile_pool(name="pers", bufs=1))
        khl = [pers.tile([128, T], BF16, name=f"khl{h}") for h in range(NH)]
        kha = [pers.tile([65, T], BF16, name=f"kha{h}") for h in range(NH)]
        # kh duplicated onto partitions 64:128 (no-lora only): lets m-hat
        # matmuls run as row-tiled PAIRS — two K=64 matmuls in PE row groups
        # (0,0)/(64,0) execute concurrently, halving the m-hat pass's time
        khh = (None if lora else
               [pers.tile([128, T], BF16, name=f"khh{h}") for h in range(NH)])
        qhh = [pers.tile([128, T], BF16, name=f"qhh{h}") for h in range(NH)]
        qla = [pers.tile([65, T], BF16, name=f"qla{h}") for h in range(NH)]
        v16 = [pers.tile([128, NH * VW], BF16, name=f"v16_{i}") for i in range(TC)]
        ident = pers.tile([128, 128], BF16, name="ident")
        ctxT_t = [pers.tile([128, T], BF16, name=f"ctxT{c}") for c in range(2)]

        # ---------------- attention-lifetime pools ----------------
        # (ptp/ost_p and the PSUM pools enter after phase 1 so their space
        # reuses the x tiles' / projection pools')
        att = ctx.enter_context(tc.tile_pool(name="att", bufs=2))
        drp = ctx.enter_context(tc.tile_pool(name="drp", bufs=2, space="DRAM"))

        woT_t = [pers.tile([128, E], BF16, name=f"woT{cc}") for cc in range(2)]

        # ---------------- m-hat machinery (emitted interleaved) ----------
        # mh_step(h, i, pool), i in 0..63: one single-bf16 [q,k] score matmul
        # (qt = i//4 stationary, k-slice i%4 moving) + row-max reduce.
        # mh_finish(h, pool): merge quarter maxes, negate, PE-transpose, DRAM
        # bounce into qla[h] row 64.
        rm4 = {}

        def mh_step(h, i, pool):
            qt, quarter = i // 4, i % 4
            if quarter == 0 and qt == 0:
                rm4[h] = [att.tile([128, 16], F32, tag=f"rm4{q}", name=f"rm4_{h}{q}")
                          for q in range(4)]
            if lora or i % 2 == 0:
                ms = pool.tile([128, 512], F32, tag="ms", name="ms")
                nc.tensor.matmul(ms, qhh[h][0:64, ts(qt, 128)],
                                 khl[h][0:64, ts(quarter, 512)],
                                 start=True, stop=True)
            else:
                # odd steps ride PE row group (64,0) concurrently with the
                # preceding even step's matmul in rows 0:63
                ms = pool.tile([128, 512], F32, tag="ms", name="ms")
                nc.tensor.matmul(ms, qhh[h][64:128, ts(qt, 128)],
                                 khh[h][64:128, ts(quarter, 512)],
                                 start=True, stop=True)
            nc.vector.reduce_max(out=rm4[h][quarter][:, qt:qt + 1], in_=ms, axis=AX.X)

        rm16s_t = {}

        def mh_merge(h):
            # DVE-only: merge quarter maxes and negate
            r = rm4[h]
            ra = att.tile([128, 16], F32, name="ra")
            rb = att.tile([128, 16], F32, name="rb")
            nc.vector.tensor_max(ra, r[0], r[1])
            nc.vector.tensor_max(rb, r[2], r[3])
            rm16 = att.tile([128, 16], F32, name="rm16")
            nc.vector.tensor_max(rm16, ra, rb)
            rm16s = att.tile([128, 16], BF16, tag="rm16s", name="rm16s")
            nc.vector.tensor_scalar_mul(rm16s, rm16, -1.0)
            rm16s_t[h] = rm16s

        def mh_bounce(h, pool):
            # transpose on PE, then a burst-contiguous DRAM bounce:
            # qla[h][64, qt*128+q] = rm16s[q, qt]  (emitted well after
            # mh_merge so the PE transpose never waits on the DVE chain)
            mtr = pool.tile([16, 128], BF16, tag="ms", name="mtr")
            nc.tensor.transpose(mtr, rm16s_t[h], ident)
            rmT = att.tile([16, 128], BF16, name="rmT")
            nc.vector.tensor_copy(rmT, mtr)
            dr = drp.tile([16, 128], BF16, name="mh_dr")
            nc.sync.dma_start(out=dr, in_=rmT)
            src = bass.AP(tensor=dr.tensor, offset=dr.offset, ap=[[1, 16 * 128]])
            nc.sync.dma_start(out=qla[h][64:65, :], in_=src)

        def mh_finish(h, pool):
            mh_merge(h)
            mh_bounce(h, pool)

        # ---------------- phase 1: projections ----------------
        with ExitStack() as ph1:
            ld = ph1.enter_context(tc.tile_pool(name="ld", bufs=1))
            wpool = ph1.enter_context(
                tc.tile_pool(name="wpool", bufs=1 if lora else 2))
            pps = ph1.enter_context(
                tc.tile_pool(name="pps", bufs=1 if lora else 2, space="PSUM"))
            msp0 = ph1.enter_context(tc.tile_pool(name="msp0", bufs=4, space="PSUM"))
            if lora:
                upsp = ph1.enter_context(
                    tc.tile_pool(name="upsp", bufs=1, space="PSUM"))

            # pre-warm the PE during the initial DMA wait: the scratch memset
            # goes on GpSimd (whose queue is empty at t=0 — DVE's is not), so
            # the dependency-free dummy matmuls run from ~2us and flip the HAM
            # clock gate to 8/8 before the first projection group
            scr = ld.tile([128, 640], BF16, name="scr")
            nc.gpsimd.memset(scr, 0.0)
            for _ in range(22):
                ms = msp0.tile([128, 512], F32, tag="ms", name="warm")
                nc.tensor.matmul(ms, scr[:, 0:128], scr[:, 128:640],
                                 start=True, stop=True)

            for h in range(NH):
                nc.vector.memset(kha[h][64:65, :], 1.0)
            for tci in range(TC):
                nc.vector.memset(v16[tci], 1.0)



            # one consolidated DMA per (weight, half): [128, ci, OL] pattern
            wt = {}

            # hi-half DMAs go on the Sync queue, lo-half on GpSimd's, so the
            # two serial DMA chains run concurrently and the first projection
            # group (wh·xh) is gated only by the short Sync chain
            def load_w_half(p, s):
                wa = wpool.tile([128, CI, OL], BF16, tag=f"w{s}", name=f"w{p}{s}")
                src = bass.AP(tensor=w_d[p + s], offset=0,
                              ap=[[OL, 128], [128 * OL, CI], [1, OL]])
                nc.sync.dma_start(out=wa, in_=src)
                return wa

            def load_w(p):
                wt[p] = [load_w_half(p, "h"),
                         None if p == "v" else load_w_half(p, "l")]

            # x arrives in ns-major 512-column slices (one DMA per slice
            # covering all 16 row-chunks). DMAs are serial on the Sync queue,
            # so issue exactly in first-use order: wq-hi, x-hi ns0 (the first
            # projection group's 8 wh·xh matmuls), then wq-lo, x-lo ns0, ...
            xth_a = ld.tile([128, CI, T], BF16, name="xth_a")
            xtl_a = ld.tile([128, CI, T], BF16, name="xtl_a")

            def load_x(dst, dram, ns, half=None):
                ci0, cin = (0, CI) if half is None else (half * CI // 2, CI // 2)
                src = bass.AP(tensor=dram, offset=ci0 * 128 * T + ns * 512,
                              ap=[[T, 128], [128 * T, cin], [1, 512]])
                nc.sync.dma_start(out=dst[:, ci0:ci0 + cin, ts(ns, 512)], in_=src)

            # DMA issue in exact first-use order (serial Sync queue): wq-hi,
            # x-hi ns0 (halved so the first 4 matmuls start sooner), x-lo ns0,
            # then wq-lo (its wl·xh consumers come after the xl ones)
            wqh = load_w_half("q", "h")
            load_x(xth_a, xth_d, 0, half=0)
            load_x(xth_a, xth_d, 0, half=1)
            load_x(xtl_a, xtl_d, 0, half=0)
            load_x(xtl_a, xtl_d, 0, half=1)
            wql = load_w_half("q", "l")
            wt["q"] = [wqh, wql]
            for ns in range(1, NS):
                load_x(xth_a, xth_d, ns)
                load_x(xtl_a, xtl_d, ns)
            xth_t = [xth_a[:, ci, :] for ci in range(CI)]
            xtl_t = [xtl_a[:, ci, :] for ci in range(CI)]

            u_bf = {}
            b_t = {}
            if lora:
                # single-bf16 LoRA: A/B factors are tiny additive corrections,
                # so one bf16 pass is well within the error budget
                ah_t = []
                for ci in range(CI):
                    t_ = ld.tile([128, 3 * R], BF16, name=f"ah{ci}")
                    nc.sync.dma_start(out=t_, in_=ah_d[ts(ci, 128), :])
                    ah_t.append(t_)
                for key2, d in b_d.items():
                    t_ = ld.tile([R, OL], BF16, name=f"b{key2}")
                    nc.sync.dma_start(out=t_, in_=d[:, :])
                    b_t[key2] = t_

                # u_all = x @ A_all, shared M=24 pass
                uf = ld.tile([3 * R, T], F32, name="uf")
                for ns in range(NS):
                    sl = ts(ns, 512)
                    ups = upsp.tile([3 * R, 512], F32, tag="ups", name="ups")
                    n_mm = 2 * CI
                    i = 0
                    for ci in range(CI):
                        for x_t in (xth_t[ci], xtl_t[ci]):
                            nc.tensor.matmul(ups, ah_t[ci], x_t[:, sl],
                                             start=(i == 0), stop=(i == n_mm - 1))
                            i += 1
                    nc.any.tensor_copy(uf[:, sl], ups)
                for pi, p in enumerate("qkv"):
                    upf = ld.tile([R, T], F32, tag="upf", name=f"u{p}f")
                    nc.sync.dma_start(out=upf, in_=uf[pi * R:(pi + 1) * R, :])
                    uh = ld.tile([R, T], BF16, name=f"u{p}h")
                    nc.vector.tensor_copy(uh, upf)
                    u_bf[p] = uh

            # --- q/k projections, transposed layout [OL, T] ---
            # MM sequence is kind-major (wh·xh first) so the first group can
            # start before xtl's DMA lands
            def qk_proj(p, oc, mh_per_ns=0, mh_base=0):
                wh_a, wl_a = wt[p]
                osl = slice(oc * 128, oc * 128 + 128)
                h0, h1 = 2 * oc, 2 * oc + 1
                for ns in range(NS):
                    sl = ts(ns, 512)
                    ps = pps.tile([128, 512], F32, tag="proj", name="proj")
                    seq = [(wh_a[:, ci, osl], xth_t[ci][:, sl]) for ci in range(CI)]
                    seq += [(wh_a[:, ci, osl], xtl_t[ci][:, sl]) for ci in range(CI)]
                    seq += [(wl_a[:, ci, osl], xth_t[ci][:, sl]) for ci in range(CI)]
                    if lora:
                        seq += [(b_t[p][:, osl], u_bf[p][:, sl])]
                    for i, (a, b_) in enumerate(seq):
                        nc.tensor.matmul(ps, a, b_, start=(i == 0),
                                         stop=(i == len(seq) - 1))
                    if p == "q":
                        for h, rows in ((h0, ps[0:64, :]), (h1, ps[64:128, :])):
                            nc.any.tensor_copy(qhh[h][0:64, sl], rows)
                            nc.any.tensor_copy(qhh[h][64:128, sl], rows)
                            nc.vector.tensor_sub(qla[h][0:64, sl], rows,
                                                 qhh[h][0:64, sl])
                    else:
                        for h, rows in ((h0, ps[0:64, :]), (h1, ps[64:128, :])):
                            nc.any.tensor_copy(khl[h][0:64, sl], rows)
                            nc.any.tensor_copy(kha[h][0:64, sl], rows)
                            nc.vector.tensor_sub(khl[h][64:128, sl], rows,
                                                 khl[h][0:64, sl])
                            if not lora:
                                nc.any.tensor_copy(khh[h][64:128, sl], rows)
                    for j in range(mh_per_ns):
                        mh_step(0, mh_base + ns * mh_per_ns + j, msp0)

            if lora:
                # wpool bufs=1 in the lora build: finish all readers of a
                # weight before loading the next (else buffer-rotation WAR
                # deadlocks against the in-order PE queue)
                qk_proj("q", 0)
                qk_proj("q", 1)
                load_w("k")
                qk_proj("k", 0)
                qk_proj("k", 1, mh_per_ns=8, mh_base=0)
                load_w("v")
                v_mh, v_base = 2, 32
            else:
                qk_proj("q", 0)
                load_w("k")
                qk_proj("k", 0)
                # mh(0) needs qhh[0]/khl[0] (ready after q/k oc0): spread its
                # 64 steps through q-oc1 (32), k-oc1 (24) and v tci 0..7 (8)
                # so the DVE reduce chain finishes alongside the v projection
                qk_proj("q", 1, mh_per_ns=8, mh_base=0)
                qk_proj("k", 1, mh_per_ns=6, mh_base=32)
                load_w("v")
                v_mh, v_base = 2, 56

            # ident / woT are needed late; issue after all projection weights
            # (runs in both lora and no-lora builds)
            nc.sync.dma_start(out=ident, in_=ident_d[:, :])
            for cc in range(2):
                nc.sync.dma_start(out=woT_t[cc], in_=woT_d[ts(cc, 128), :])

            # --- v directly in [T, O] orientation (no transposes) ---
            wvh = wt["v"][0]
            for tci in range(TC):
                tsl = ts(tci, 128)
                ps = pps.tile([128, OL], F32, tag="proj", name="proj")
                seq = [(xth_t[ci][:, tsl], wvh[:, ci, :]) for ci in range(CI)]
                if lora:
                    seq += [(u_bf["v"][:, tsl], b_t["v"][:, :])]
                for i, (a, b_) in enumerate(seq):
                    nc.tensor.matmul(ps, a, b_, start=(i == 0),
                                     stop=(i == len(seq) - 1))
                for h in range(NH):
                    nc.any.tensor_copy(v16[tci][:, h * VW:h * VW + 64],
                                       ps[:, h * 64:(h + 1) * 64])
                i_mh = v_base + tci * v_mh
                for j in range(v_mh):
                    if i_mh + j < 64:
                        mh_step(0, i_mh + j, msp0)
                if i_mh + v_mh >= 64 and i_mh < 64:
                    mh_finish(0, msp0)

        # ---------------- phase 3: attention ----------------
        ptp = ctx.enter_context(tc.tile_pool(name="ptp", bufs=2))
        ost_p = ctx.enter_context(tc.tile_pool(name="ost", bufs=3))
        sps = ctx.enter_context(
            tc.tile_pool(name="sps", bufs=2 if lora else 3, space="PSUM"))
        msp = ctx.enter_context(tc.tile_pool(name="msp", bufs=4, space="PSUM"))
        cps = ctx.enter_context(tc.tile_pool(name="cps", bufs=1, space="PSUM"))

        # outproj(tci): emitted interleaved into head 3's loop
        ops_state = {}

        def outproj_mm(tci, no):
            tsl = ts(tci, 128)
            op_t = msp.tile([128, 512], F32, tag="ms", name="op")
            for cc in range(2):
                nc.tensor.matmul(op_t, ctxT_t[cc][:, tsl], woT_t[cc][:, ts(no, 512)],
                                 start=(cc == 0), stop=(cc == 1))
            if no == 0:
                ops_state[tci] = ost_p.tile([128, E], BF16, tag="ost", name="ost")
            ost = ops_state[tci]
            nc.vector.tensor_copy(ost[:, ts(no, 512)], op_t)
            # per-half store so the final DMA isn't gated on both casts
            nc.sync.dma_start(out=outp_d[tsl, ts(no, 512)], in_=ost[:, ts(no, 512)])

        def outproj_steps(qb):
            # 8 paired-MM slots per score loop: 4 tci x 2 no
            return [(tci, no) for tci in range(qb * 4, qb * 4 + 4)
                    for no in range(2)]

        for h in range(NH):
            ch = h // 2
            pr = (h % 2) * 64
            mh_count = [0]
            for qb in range(NS):
                qsl = ts(qb, 512)
                # --- sT pass: K-stacked scores with fused -m-hat -> exp ---
                # one mh(h+1) step (or outproj MM for h==3) per kt so the PE
                # queue never stalls behind the DVE reduce chain
                steps = outproj_steps(qb - 1) if (h == 3 and qb > 0) else None
                pT = [ptp.tile([128, 512], BF16, tag=f"pt{i}", name=f"pt{i}")
                      for i in range(TC)]
                cxa = cps.tile([VW, 512], F32, tag="cxa", name="cxa")

                def pv_mm(kt):
                    nc.tensor.matmul(cxa, v16[kt][:, h * VW:(h + 1) * VW], pT[kt],
                                     start=(kt == 0), stop=(kt == TC - 1),
                                     skip_group_check=True)

                for kt in range(TC):
                    st = sps.tile([128, 512], F32, tag="st", name="st")
                    # kh·qh + kl·qh in one K=128 matmul (qh duplicated)
                    nc.tensor.matmul(st, khl[h][:, ts(kt, 128)], qhh[h][:, qsl],
                                     start=True, stop=False)
                    # kh·ql + ones·(-m-hat), K=65
                    nc.tensor.matmul(st, kha[h][:, ts(kt, 128)],
                                     qla[h][:, qsl], start=False, stop=True)
                    nc.scalar.activation(out=pT[kt], in_=st, func=Exp, scale=0.125)
                    # PV interleaved at lag 2 so cxa completes with the score
                    # loop and the normalize chain starts immediately
                    if kt >= 2:
                        pv_mm(kt - 2)
                    if h < NH - 1 and qb < 3:
                        # 64 mh(h+1) steps over qb0..2 so the bounce lands
                        # during qb3, before head h+1 needs it; no-lora emits
                        # row-tiled pairs back-to-back so they overlap on PE
                        target = ((qb * TC + kt + 1) * 64) // (3 * TC)
                        if not lora:
                            target &= ~1
                        while mh_count[0] < target:
                            mh_step(h + 1, mh_count[0], msp)
                            mh_count[0] += 1
                    elif steps is not None and kt % 2 == 0:
                        outproj_mm(*steps[kt // 2])
                    if h < NH - 1 and qb == 3 and kt == 8:
                        mh_bounce(h + 1, msp)
                pv_mm(TC - 2)
                pv_mm(TC - 1)
                if h == NH - 1 and qb == NS - 1:
                    # keep the PE busy through the final normalize chain
                    # (a >3.4us idle window re-throttles the HAM clock gate,
                    # halving the closing output-projection matmuls)
                    for _ in range(10):
                        ms = msp.tile([128, 512], F32, tag="ms", name="fill")
                        nc.tensor.matmul(ms, khl[0][:, 0:128], qhh[0][:, 0:512],
                                         start=True, stop=True)
                if h < NH - 1 and qb == 2:
                    mh_merge(h + 1)
                # evacuate PSUM immediately so the next PV group never waits
                cxs = att.tile([VW, 512], F32, tag="cxs", name="cxs")
                nc.vector.tensor_copy(cxs, cxa)
                # --- normalize by Z (row 64) off the critical path ---
                zrow = att.tile([1, 512], F32, name="zrow")
                nc.vector.tensor_copy(zrow, cxs[64:65, :])
                rcpz = att.tile([1, 512], F32, name="rcpz")
                nc.vector.reciprocal_approx_fast(out=rcpz, in_=zrow)
                rcp_bc = att.tile([64, 512], F32, name="rcp_bc")
                nc.gpsimd.partition_broadcast(rcp_bc, rcpz, channels=64)
                nc.vector.tensor_mul(ctxT_t[ch][pr:pr + 64, qsl], cxs[0:64, :],
                                     rcp_bc)

        # ---------------- tail: last output-projection block ----------------
        for tci, no in outproj_steps(NS - 1):
            outproj_mm(tci, no)

    nc.compile()
    _CACHE[key] = nc
    return nc


def _split(a):
    h = a.astype(bf16)
    l = (a - h.astype(np.float32)).astype(bf16)
    return h, l


def _shard(inputs, lora):
    x = np.asarray(inputs["x"], np.float32)
    Wo = np.asarray(inputs["Wo"], np.float32)
    ident = np.eye(128, dtype=np.float32).astype(bf16)
    if lora:
        A_all = np.concatenate([np.asarray(inputs["Aq"], np.float32),
                                np.asarray(inputs["Ak"], np.float32),
                                np.asarray(inputs["Av"], np.float32)], axis=1)
        ah = A_all.astype(bf16)
    in_maps = []
    for core in range(8):
        b, hp = core // 4, core % 4
        o0 = hp * OL
        xT = np.ascontiguousarray(x[b].T)
        xh, xl = _split(xT)
        m = {"xth": xh, "xtl": xl, "ident": ident}
        for p in "qkv":
            W = np.asarray(inputs["W" + p], np.float32)
            Ws = np.ascontiguousarray(W[o0:o0 + OL, :].T)
            wh, wl = _split(Ws)
            m["w%sh" % p] = wh
            if p != "v":
                m["w%sl" % p] = wl
            if lora:
                B = np.asarray(inputs["B" + p], np.float32)[:, o0:o0 + OL] * 2.0
                m["b" + p] = B.astype(bf16)
        m["woT"] = np.ascontiguousarray(Wo[:, o0:o0 + OL].T).astype(bf16)
        if lora:
            m["ah"] = ah
        in_maps.append(m)
    return in_maps


def _run(inputs, trace=False, **kw):
    lora = not all(
        np.count_nonzero(np.asarray(inputs["B" + p])) == 0 for p in "qkv")
    nc = _build(lora)
    in_maps = _shard(inputs, lora)
    res = run_bass_kernel_spmd(nc, in_maps, core_ids=list(range(8)), trace=trace, **kw)
    bo = np.asarray(inputs["bo"], np.float32)
    parts = [res.results[c]["outp"].astype(np.float64) for c in range(8)]
    out = np.stack([sum(parts[0:4]), sum(parts[4:8])]) + bo.astype(np.float64)
    return out.astype(np.float32), res


def kernel(**inputs):
    out, _ = _run(inputs)
    return out
